# revision 53
# baseline (speedup 1.0000x reference)
"""Trainium2 Bass kernel for nn_AttnBlock (B=16, C=512, H=W=32).

Strategy (v7: Wov-fold + fp8-DR scores/proj + transpose attention)
------------------------------------------------------------------
Data-parallel over batch: 16 batch elements / 8 NeuronCores = 2 per core.

The q/k projections never materialize: scores s[i,j] = hn_i^T M hn_j with
M = Wq^T Wk precomputed in f64 on the host; M is held bf16 (0.4% rel err,
half the DMA).  The v/proj GEMMs fuse too: out = Wo(Wv(hn @ attn)) =
Wov @ (hn @ attn) with Wov = Wo Wv precomputed f64 host-side, so the v
GEMM disappears; the attention aggregation ha[c,i] = sum_j hn[c,j]eT[j,i]
uses a PE-transposed fp8 copy of hn as the DoubleRow stationary (fp8
transposes stream 128 rows/block, 4x cheaper than the v GEMM they
replace).

fp8 e4m3 budget (sim rel err 1.41e-2 vs the 2e-2 gate): scores run
DoubleRow with u8 = fp8(M hn) stationary and hn8 moving; eT = exp(s-3.5)
fp8; ha DR with hn8T stationary; proj DR with Wov8 = fp8(4 Wov) and
ha8 = fp8(ha/(4r)) - the 1/4 rides the existing exp(-ln r) bias so no
extra eviction op.  The u GEMM stays bf16xbf16 (score accuracy anchor).

Pipeline: groupnorm joins run per channel-tile so u matmuls start as
soon as (x[ct], MT) land; batch-0 x spreads over 5 DMA queues with MT
first on its own queue; f32 warm-up matmuls on arriving x chunks keep
the PE DVFS p-state ramped; y writeback rotates DMA queues so the tail
drains in parallel.  GroupNorm rstd = exp(-0.5 ln(var+eps)) keeps ACT
on the ln/exp table (no Sqrt reload).  The fp8 row-sum uses an all-ones
[128,2,128] DoubleRow stationary broadcasting sum_j eT[j,i] to all
partitions.
"""
import contextlib
import os
import sys

for _p in ("/opt/trn_rl_repo",):
    if _p not in sys.path and os.path.isdir(_p):
        sys.path.append(_p)

import ml_dtypes
import numpy as np

import concourse.bass as bass
import concourse.tile as tile
from concourse import mybir
from concourse.bass_utils import run_bass_kernel_spmd
from concourse.vector_clock import ScopedClock

F32 = mybir.dt.float32
F32R = mybir.dt.float32r
BF16 = mybir.dt.bfloat16
FP8 = mybir.dt.float8e4
NP_FP8 = ml_dtypes.float8_e4m3
NP_BF16 = ml_dtypes.bfloat16
AF = mybir.ActivationFunctionType
ALU = mybir.AluOpType
DR = mybir.MatmulPerfMode.DoubleRow

NCORES = 8
B, C, N = 16, 512, 1024
H = W = 32
NB = B // NCORES          # batch elements per core
CT = C // 128             # channel tiles of 128
NT = N // 128             # pixel tiles of 128
IC = N // 512             # query chunks of 512
KP = CT // 2              # DoubleRow channel-tile pairs
JP = NT // 2              # DoubleRow pixel-tile pairs
G, GS = 32, 16            # groups, channels per group
GPT = 128 // GS           # groups per 128-channel tile
EPS = 1e-6
EXP_BIAS = -3.5           # exp(s + EXP_BIAS): keeps eT below e4m3 max 240
SPLIT = 4.0               # Wov8 = fp8(4 Wov); 1/4 folded into exp(-ln r)
F8MAX = 240.0


class _TC(tile.TileContext):
    """TileContext with multi-wait instructions split for this walrus.

    The pinned walrus accepts at most one semaphore wait per instruction
    (two for EventSemaphore).  Tile's scheduler can attach several; the
    extras are moved onto no-op carriers committed immediately before on
    the same engine, which is semantically identical (engine streams are
    sequential).
    """

    def _commit_instruction(self, inst, lazy_reg_writes: bool = True):
        si = inst.sync_info
        cap = 2 if isinstance(inst, mybir.InstEventSemaphore) else 1
        if si is not None and si.on_wait and len(si.on_wait) > cap and \
                inst.engine != mybir.EngineType.Unassigned:
            waits = list(si.on_wait)
            inst.sync_info = mybir.SyncInfo(
                on_wait=waits[:cap], on_update=list(si.on_update or [])
            )
            for w in waits[cap:]:
                nop = mybir.InstNoOp(
                    name=self.nc.get_next_instruction_name(),
                    ins=[],
                    outs=[],
                    engine=inst.engine,
                    sync_info=mybir.SyncInfo(on_wait=[w], on_update=[]),
                    bass_nofuse=True,
                )
                super()._commit_instruction(nop, lazy_reg_writes=False)
        super()._commit_instruction(inst, lazy_reg_writes)

    def _drain_and_barrier(self, tick_clock, wait_clock):
        # Collect the final-tick waits on a probe drain, then distribute
        # them across all engines (one wait per carrier instruction).
        # Each engine then signals a star-barrier semaphore; gpsimd
        # collects all signals and clears the semaphores.  This replaces
        # Tile's two EVSEM-butterfly all-engine barriers (~10us).
        nc = self.nc
        drain_inst = nc.sync.drain()
        wait_clock.add_sem_waits(
            drain_inst.ins, ScopedClock({None: tick_clock.global_clock})
        )
        si = drain_inst.ins.sync_info
        waits = list(si.on_wait) if si and si.on_wait else []
        drain_inst.ins.sync_info = mybir.SyncInfo(
            on_wait=waits[:1], on_update=[]
        )
        engines = list(nc.engines.values())
        for i, w in enumerate(waits[1:]):
            eng = engines[i % len(engines)]
            nop = eng.nop(nofuse=True)
            nop.ins.sync_info = mybir.SyncInfo(on_wait=[w], on_update=[])
        star = nc.alloc_semaphore("tile_star_barrier")
        nsig = 0
        for eng in engines:
            if eng is not nc.gpsimd:
                eng.sem_inc(star, 1)
                nsig += 1
        nc.gpsimd.wait_ge(star, nsig)
        assert self.sems is not None
        popped = nc._tile_sem_poison_stack.pop()
        assert popped is self._sem_poison
        nc.clear_and_free_semaphores(
            list(self.sems.allocated().values()) + [star])


def build_nc(use_bq: bool, use_bk: bool, use_bv: bool, use_bo: bool):
    # bq/bk fold into bu = Wk^T bq + Wq^T bk (added to u); bv/bo fold
    # into bvo = Wo bv + bo (added at proj, attn rows sum to 1).
    nc = bass.Bass()

    x_d = nc.declare_dram_parameter("x", [NB, 128, CT, N], F32, isOutput=False)
    y_d = nc.declare_dram_parameter("y", [NB, 128, CT, N], F32, isOutput=True)
    MT_d = nc.declare_dram_parameter("MTb", [128, CT, 512], BF16,
                                     isOutput=False)
    wov_d = nc.declare_dram_parameter("wov8T", [128, CT, 512], FP8,
                                      isOutput=False)
    # pk1 packs [R | nsc | nbi | bu] columns (f32r) where R = S S^T/GS is
    # the fused group-average+broadcast matrix; pk2 packs the f32r row
    # constants [ones512 | bvo].
    pk1_d = nc.declare_dram_parameter("pk1", [128, 128 + 3 * CT], F32R,
                                      isOutput=False)
    pk2_d = nc.declare_dram_parameter("pk2", [1, 2 * 512], F32R,
                                      isOutput=False)

    scale = float(C) ** -0.5
    use_bu = use_bq or use_bk
    use_bvo = use_bv or use_bo

    with _TC(nc) as tc:
        with (
            tc.tile_pool(name="consts", bufs=1) as consts,
            tc.tile_pool(name="big", bufs=1) as big,
            tc.tile_pool(name="small", bufs=2) as small,
            tc.tile_pool(name="psum", bufs=1, space="PSUM") as psum,
        ):
            # ---- input DMA spread: ct0 leads on sync+scalar, MT heads
            # its own queue (first u GEMM gates on it), ct3 on the PE
            # queue (idle until ~15us anyway), small consts on vector.
            # x chunks lead every queue so the per-ct groupnorm chains
            # unblock in arrival order; gpsimd fronts MTb (first u GEMM
            # gates on it) + the stats constants, then takes ct3; id8
            # (transposes, ~17us) and wov8T (proj, ~45us) trail there.
            pk1_sb = consts.tile([128, 128 + 3 * CT], F32R, tag="pk1")
            nc.gpsimd.dma_start(out=pk1_sb, in_=pk1_d[:, :])
            MT_sb = consts.tile([128, CT, 512], BF16, tag="MTb", name="w_MTb")
            nc.gpsimd.dma_start(out=MT_sb[:, 0:2], in_=MT_d[:, 0:2, :])
            x_sb0 = big.tile([128, CT, N], F32, tag="x", bufs=2, name="x_sb0")
            x_engs = [nc.sync, nc.scalar, nc.sync, nc.scalar,
                      nc.sync, nc.scalar, nc.gpsimd, nc.gpsimd]
            x_chunks = [(0, 0), (0, 1), (1, 0), (1, 1),
                        (2, 0), (2, 1), (3, 0), (3, 1)]
            for eng, (ct, h) in zip(x_engs, x_chunks):
                if (ct, h) == (3, 1):
                    # second MTb half rides between the ct3 chunks (only
                    # u-ct2+ needs it, ~18us)
                    nc.gpsimd.dma_start(out=MT_sb[:, 2:4], in_=MT_d[:, 2:4, :])
                eng.dma_start(
                    out=x_sb0[:, ct, h * 512:(h + 1) * 512],
                    in_=x_d[0, :, ct, h * 512:(h + 1) * 512])
            wov_sb = consts.tile([128, CT, 512], FP8, tag="wov8T",
                                 name="w_wov8T")
            nc.gpsimd.dma_start(out=wov_sb, in_=wov_d[:, :, :])
            pk2_sb = consts.tile([1, 2 * 512], F32R, tag="pk2")
            nc.scalar.dma_start(out=pk2_sb, in_=pk2_d[:, :])
            R_sb = pk1_sb[:, 0:128]
            nsc_sb = pk1_sb[:, 128:128 + CT]
            nbi_sb = pk1_sb[:, 128 + CT:128 + 2 * CT]
            bu_sb = pk1_sb[:, 128 + 2 * CT:128 + 3 * CT]
            ones512_sb = pk2_sb[:, 0:512]
            bvo_sb = pk2_sb[:, 512:1024]
            # All-ones DoubleRow stationary [128, 2, 128]: the row-sum matmul
            # broadcasts sum_j eT[j, i] into every output partition (dual-fp8
            # LDWEIGHTS forbids narrow M; M=128 is legal and the broadcast
            # replaces a separate ones1 outer-product matmul).
            ones2_sb = consts.tile([128, 2, 128], FP8, tag="ones2")
            nc.gpsimd.memset(ones2_sb, 1.0)
            eps_sb = consts.tile([128, 1], F32, tag="eps")
            nc.vector.memset(eps_sb, EPS)
            ebias_sb = consts.tile([128, 1], F32, tag="ebias")
            nc.vector.memset(ebias_sb, EXP_BIAS)
            # rinv bias: exp(-ln r - ln SPLIT) = 1/(SPLIT*r)
            rbias_sb = consts.tile([128, 1], F32, tag="rbias")
            nc.vector.memset(rbias_sb, -float(np.log(SPLIT)))
            # Warm the ACT ln/exp table while DMAs stream, so the first
            # groupnorm join / exp does not pay the table load.
            warm_sb = consts.tile([128, 1], F32, tag="warm")
            nc.scalar.activation(out=warm_sb, in_=eps_sb, func=AF.Ln,
                                 bias=eps_sb, scale=1.0)
            nc.scalar.activation(out=warm_sb, in_=warm_sb, func=AF.Exp,
                                 scale=-0.5)

            for b in range(NB):
                # ---- load x ----
                if b == 0:
                    x_sb = x_sb0
                else:
                    x_sb = big.tile([128, CT, N], F32, tag="x", bufs=2,
                                    name=f"x_sb{b}")
                    x1_engs = [nc.sync, nc.gpsimd, nc.scalar, nc.sync]
                    for ct in range(CT):
                        x1_engs[ct].dma_start(out=x_sb[:, ct],
                                              in_=x_d[b, :, ct])

                # ---- GroupNorm, fully per channel tile (each group of 16
                # channels lives inside one 128-row tile, so ct pipelines
                # independently behind its own DMA chunks).  The
                # tile_wait_until stagger mirrors real HBM arrival times:
                # the scheduler's sim has near-instant DMA, which otherwise
                # bakes all bn_stats ahead of the join chains in the DVE
                # stream and stalls the first u GEMM ~10us.
                # per-ct tiles: DMA reads are dependency-tracked at tile
                # granularity, so a single [128,CT,N] tile would make the
                # ct0 transpose wait for hnb ct3
                hnb_cts = [big.tile([128, N], BF16, tag=f"hnb{ct}", bufs=2,
                                    name=f"hnb_sb{b}_{ct}")
                           for ct in range(CT)]
                hn8_sb = big.tile([128, CT, N], FP8, tag="hn8", bufs=2,
                                  name=f"hn8_sb{b}")
                # ct-major so each per-ct DMA transpose writes a contiguous
                # [128, NT, 128] block (strided xbar destinations are
                # broken on hw); the DR stationary slice [j, jt-pair, cc]
                # works the same from this layout.
                hnbT_sb = big.tile([128, CT, NT, 128], BF16, tag="hnbT",
                                   bufs=2, name=f"hnbT_sb{b}")
                hn8T_sb = big.tile([128, CT, NT, 128], FP8, tag="hn8T",
                                   bufs=2, name=f"hn8T_sb{b}")
                # u PSUM tiles for ots 0-2 are held across the gn loop so
                # each ct's contribution accumulates as soon as its hnb
                # lands (emitting u after the loop would queue every
                # stats/broadcast matmul ahead of it on the in-order PE).
                u8_sb = big.tile([128, CT, N], FP8, tag="u8", bufs=2,
                                 name=f"u8_sb{b}")
                u_ps = [psum.tile([128, 2 * 512], F32, tag="mm", bufs=3,
                                  name=f"u_ps_{b}_{ot}") for ot in range(3)]
                ct_wait = ((0.0115, 0.0145, 0.0175, 0.0205) if b == 0
                           else (0.024, 0.026, 0.028, 0.030))
                for ct in range(CT):
                  with tc.tile_wait_until(ct_wait[ct]):
                    stats = small.tile([128, 2, 6], F32, tag=f"bnst{ct}",
                                       name=f"bnst_{b}_{ct}")
                    ts = small.tile([128, 2], F32R, tag=f"ts{ct}",
                                    name=f"ts_{b}_{ct}")
                    mv = small.tile([128, 2], F32, tag=f"mv{ct}",
                                    name=f"mv_{b}_{ct}")
                    for h in range(2):
                        nc.vector.bn_stats(
                            out=stats[:, h],
                            in_=x_sb[:, ct, h * 512:(h + 1) * 512],
                        )
                        if b == 0 and ct == 0:
                            # PE warm-up: f32 matmul on the arriving chunk
                            # keeps the DVFS p-state ramping; never read.
                            wps = psum.tile([128, 256], F32, tag="small",
                                            bufs=2, name=f"warm_ps_{ct}_{h}")
                            nc.tensor.matmul(
                                wps,
                                lhsT=x_sb[:, ct, h * 512:h * 512 + 128],
                                rhs=x_sb[:, ct, h * 512:h * 512 + 256],
                                start=True, stop=True, skip_group_check=True,
                            )
                    nc.vector.bn_aggr(out=mv, in_=stats)
                    nc.vector.tensor_copy(ts[:, 0:1], mv[:, 0:1])
                    nc.vector.tensor_mul(ts[:, 1:2], mv[:, 0:1], mv[:, 0:1])
                    nc.vector.tensor_add(ts[:, 1:2], ts[:, 1:2], mv[:, 1:2])
                    # fused group-average + broadcast: R = S S^T/GS gives
                    # per-channel [group-mean, group-E[x^2]] in one matmul
                    rps = psum.tile([128, 2], F32, tag="small", bufs=2,
                                    name=f"r_ps_{b}_{ct}")
                    nc.tensor.matmul(rps, lhsT=R_sb, rhs=ts,
                                     start=True, stop=True)
                    A1 = small.tile([128, 1], F32, tag=f"A{ct}",
                                    name=f"A_{b}_{ct}")
                    B1 = small.tile([128, 1], F32, tag=f"B{ct}",
                                    name=f"B_{b}_{ct}")
                    v1 = small.tile([128, 1], F32, tag=f"v{ct}",
                                    name=f"v_{b}_{ct}")
                    rsb = small.tile([128, 2], F32, tag=f"rsb{ct}",
                                     name=f"rsb_{b}_{ct}")
                    nc.vector.tensor_copy(rsb, rps)
                    nc.vector.tensor_mul(v1, rsb[:, 0:1], rsb[:, 0:1])
                    nc.vector.tensor_sub(v1, rsb[:, 1:2], v1)
                    # rstd = exp(-0.5*ln(var+eps)): stays on the ln/exp table
                    nc.scalar.activation(out=v1, in_=v1,
                                         func=AF.Ln, bias=eps_sb, scale=1.0)
                    nc.scalar.activation(out=v1, in_=v1,
                                         func=AF.Exp, scale=-0.5)
                    nc.vector.tensor_mul(A1, v1, nsc_sb[:, ct:ct + 1])
                    nc.vector.tensor_mul(B1, rsb[:, 0:1], A1)
                    nc.vector.tensor_sub(B1, nbi_sb[:, ct:ct + 1], B1)
                    # hn = x*A + B: bf16 (u GEMM moving) on GPSIMD, fp8
                    # copy (scores moving + transpose source) on DVE/ACT
                    # (GPSIMD takes the slow one; PSUM evictions need
                    # DVE/ACT anyway so keep their slack for those).
                    nc.gpsimd.tensor_scalar(
                        out=hnb_cts[ct], in0=x_sb[:, ct],
                        scalar1=A1, scalar2=B1,
                        op0=ALU.mult, op1=ALU.add,
                    )
                    # ACT, not DVE: the DVE stream carries the join
                    # chains that gate the stats matmuls - a 1.2us
                    # hn8 op there stalls the next ct's group stats.
                    # (Pool is out too: serialized behind the hnb chain,
                    # it starves the transposes.)
                    nc.scalar.activation(
                        out=hn8_sb[:, ct], in_=x_sb[:, ct],
                        func=AF.Identity, scale=A1, bias=B1,
                    )
                    # hnbT[j, jt, c] = hnb[c, jt*128+j] via the DMA xbar
                    # (SBUF->SBUF, off the HBM fabric); replaces 16 PE
                    # transposes per ct.  Sync engine only (the 1.3us
                    # trigger would eat ACT time), explicitly staggered.
                    tw = ((0.016, 0.018, 0.020, 0.0255)[ct] if b == 0
                          else 0.042)
                    with tc.tile_wait_until(tw):
                        nc.sync.dma_start_transpose(
                            hnbT_sb[:, ct], hnb_cts[ct][:, :])
                    # u = M hn contributions of this ct (ots 0-2)
                    for ot in range(3):
                        for ic in range(IC):
                            nc.tensor.matmul(
                                u_ps[ot][:, ic * 512:(ic + 1) * 512],
                                lhsT=MT_sb[:, ct, ot * 128:(ot + 1) * 128],
                                rhs=hnb_cts[ct][:, ic * 512:(ic + 1) * 512],
                                start=(ct == 0), stop=(ct == CT - 1),
                            )


                # ---- u = M hn (key side of the fused score GEMM, bf16),
                # evicted straight to fp8 for the DR score stationary ----
                def u_evict(ot, ps):
                    if use_bu:
                        nc.vector.tensor_scalar_add(
                            u8_sb[:, ot], ps, bu_sb[:, ot:ot + 1])
                    elif b == 0:
                        # DVE is free here for b0 (joins done); for b1 the
                        # DVE stream carries b0's ha8/y evictions, while
                        # ACT idles between b0's exps and b1's.
                        nc.vector.tensor_copy(u8_sb[:, ot], ps)
                    else:
                        nc.scalar.activation(out=u8_sb[:, ot], in_=ps,
                                             func=AF.Copy)

                for ot in range(3):
                    u_evict(ot, u_ps[ot])

                # ---- hn8T = fp8(hnbT) for the DR ha stationary ----
                for half in range(2):
                    src = hnbT_sb[:, 2 * half:2 * half + 2]
                    dst = hn8T_sb[:, 2 * half:2 * half + 2]
                    if b == 0:
                        nc.vector.tensor_copy(dst, src)
                    else:
                        nc.gpsimd.tensor_copy(dst, src)

                # u ot3 (second pass; all hnb ready, streams stall-free)
                ps3 = psum.tile([128, 2 * 512], F32, tag="mm", bufs=3,
                                name=f"u_ps_{b}_3")
                for ct in range(CT):
                    for ic in range(IC):
                        nc.tensor.matmul(
                            ps3[:, ic * 512:(ic + 1) * 512],
                            lhsT=MT_sb[:, ct, 3 * 128:4 * 128],
                            rhs=hnb_cts[ct][:, ic * 512:(ic + 1) * 512],
                            start=(ct == 0), stop=(ct == CT - 1),
                        )
                u_evict(3, ps3)

                # ---- scores sT[j,i] = sum_c u8[c,j] hn8[c,i] (DR) + exp ----
                eT_sb = big.tile([128, NT, N], FP8, tag="eT", bufs=2,
                                 name=f"eT_sb_{b}")
                for jt in range(NT):
                    ps = psum.tile([128, 2 * 512], F32, tag="mm", bufs=3,
                                   name=f"sc_ps_{b}_{jt}")
                    for kp in range(KP):
                        for ic in range(IC):
                            nc.tensor.matmul(
                                ps[:, ic * 512:(ic + 1) * 512],
                                lhsT=u8_sb[:, 2 * kp:2 * kp + 2,
                                           jt * 128:(jt + 1) * 128],
                                rhs=hn8_sb[:, 2 * kp:2 * kp + 2,
                                           ic * 512:(ic + 1) * 512],
                                start=(kp == 0), stop=(kp == KP - 1),
                                perf_mode=DR,
                            )
                    nc.scalar.activation(
                        out=eT_sb[:, jt], in_=ps,
                        func=AF.Exp, scale=scale, bias=ebias_sb[:, 0:1],
                    )
                # rb[p, i] = sum_j eT[j, i] broadcast into all partitions by
                # the all-ones DoubleRow stationary; 1/(SPLIT*r) =
                # exp(-ln(r) - ln SPLIT) lands straight in SBUF.
                rb_ps = psum.tile([128, 2 * 512], F32, tag="mm", bufs=3,
                                  name=f"rb_ps_{b}")
                for jp in range(JP):
                    for ic in range(IC):
                        nc.tensor.matmul(
                            rb_ps[:, ic * 512:(ic + 1) * 512],
                            lhsT=ones2_sb,
                            rhs=eT_sb[:, 2 * jp:2 * jp + 2,
                                      ic * 512:(ic + 1) * 512],
                            start=(jp == 0), stop=(jp == JP - 1),
                            perf_mode=DR,
                        )
                lnr_sb = small.tile([128, 2 * 512], F32, tag="lnr", bufs=2,
                                    name=f"lnr_{b}")
                nc.scalar.activation(out=lnr_sb, in_=rb_ps, func=AF.Ln)
                rinvb_sb = small.tile([128, 2 * 512], F32, tag="rinvb",
                                      bufs=2, name=f"rinvb_{b}")
                nc.scalar.activation(out=rinvb_sb, in_=lnr_sb, func=AF.Exp,
                                     scale=-1.0, bias=rbias_sb[:, 0:1])

                # ---- ha[c,i] = sum_j hn8T[j,c] eT[j,i], normalized to
                # ha8 = fp8(ha / (SPLIT*r)) for the DR proj ----
                ha8_sb = big.tile([128, CT, N], FP8, tag="ha8", bufs=2,
                                  name=f"ha8_{b}")
                for ct in range(CT):
                    ps = psum.tile([128, 2 * 512], F32, tag="mm", bufs=3,
                                   name=f"ha_ps_{b}_{ct}")
                    for jp in range(JP):
                        for ic in range(IC):
                            nc.tensor.matmul(
                                ps[:, ic * 512:(ic + 1) * 512],
                                lhsT=hn8T_sb[:, ct, 2 * jp:2 * jp + 2, :],
                                rhs=eT_sb[:, 2 * jp:2 * jp + 2,
                                          ic * 512:(ic + 1) * 512],
                                start=(jp == 0), stop=(jp == JP - 1),
                                perf_mode=DR,
                            )
                    nc.vector.tensor_mul(ha8_sb[:, ct], ps, rinvb_sb)

                # ---- y = Wov8 ha8 + [bvo] + x ----
                for ot in range(CT):
                    ps = psum.tile([128, 2 * 512], F32, tag="mm", bufs=3,
                                   name=f"pr_ps_{b}_{ot}")
                    for kp in range(KP):
                        for ic in range(IC):
                            nc.tensor.matmul(
                                ps[:, ic * 512:(ic + 1) * 512],
                                lhsT=wov_sb[:, 2 * kp:2 * kp + 2,
                                            ot * 128:(ot + 1) * 128],
                                rhs=ha8_sb[:, 2 * kp:2 * kp + 2,
                                           ic * 512:(ic + 1) * 512],
                                start=(kp == 0),
                                stop=(kp == KP - 1 and not use_bvo),
                                perf_mode=DR,
                            )
                    if use_bvo:
                        for ic in range(IC):
                            nc.tensor.matmul(
                                ps[:, ic * 512:(ic + 1) * 512],
                                lhsT=bvo_sb[0:1, ot * 128:(ot + 1) * 128],
                                rhs=ones512_sb, start=False, stop=True,
                            )
                    y_sb = big.tile([128, N], F32, tag="y", bufs=6,
                                    name=f"y_{b}_{ot}")
                    nc.vector.tensor_add(y_sb, ps, x_sb[:, ot])
                    if b == NB - 1:
                        # tail: split each writeback across two queues so
                        # the final transfer drains in half the time
                        e0, e1 = ((nc.sync, nc.gpsimd) if ot % 2 == 0
                                  else (nc.gpsimd, nc.scalar))
                        e0.dma_start(out=y_d[b, :, ot, 0:512],
                                     in_=y_sb[:, 0:512])
                        e1.dma_start(out=y_d[b, :, ot, 512:N],
                                     in_=y_sb[:, 512:N])
                    else:
                        dma_eng = (nc.sync, nc.gpsimd, nc.scalar, nc.sync)[
                            ot % 4]
                        dma_eng.dma_start(out=y_d[b, :, ot, :], in_=y_sb)
    return nc


_CACHE = {}


def _get_nc(use_bq=False, use_bk=False, use_bv=False, use_bo=False):
    key = (use_bq, use_bk, use_bv, use_bo)
    if key not in _CACHE:
        _CACHE[key] = build_nc(*key)
    return _CACHE[key]


def _q8(a):
    a = np.clip(np.asarray(a, dtype=np.float32), -F8MAX, F8MAX)
    return a.astype(NP_FP8)


def prepare(x, norm_scale, norm_bias, wq, bq, wk, bk, wv, bv, wo, bo):
    """Host-side prep: returns (in_maps, use_b* flags)."""
    x = np.ascontiguousarray(np.asarray(x, dtype=np.float32))
    f32 = lambda a: np.asarray(a, dtype=np.float32)
    norm_scale, norm_bias = f32(norm_scale), f32(norm_bias)
    wq, wk, wv, wo = f32(wq), f32(wk), f32(wv), f32(wo)
    bq, bk, bv, bo = f32(bq), f32(bk), f32(bv), f32(bo)

    # Fused score weights: s[i,j] = (Wq hn_i + bq).(Wk hn_j + bk)
    #   = hn_i^T M hn_j + (Wk^T bq + Wq^T bk).hn + const(dropped unless
    # only one of bq/bk is nonzero, where it is exactly zero).
    M = (wq.astype(np.float64).T @ wk.astype(np.float64)).astype(np.float32)
    bu = (wk.T.astype(np.float64) @ bq.astype(np.float64)
          + wq.T.astype(np.float64) @ bk.astype(np.float64)).astype(np.float32)
    # Fused output weights: Wo(Wv ha + bv) + bo = Wov ha + bvo (attention
    # rows sum to 1, so bv passes through the aggregation unchanged).
    Wov = (wo.astype(np.float64) @ wv.astype(np.float64)).astype(np.float32)
    bvo = (wo.astype(np.float64) @ bv.astype(np.float64)
           + bo.astype(np.float64)).astype(np.float32)

    # [C, C] w  ->  wT[c, o] arranged [p, ct, o]
    def arr_w(w):
        return np.ascontiguousarray(
            w.T.reshape(CT, 128, C).transpose(1, 0, 2))

    # [C] vec (channel-tile major) -> [p, ct]
    def arr_c(v):
        return np.ascontiguousarray(v.reshape(CT, 128).T)

    S = np.zeros((128, GPT), np.float32)
    S[np.arange(128), np.arange(128) // GS] = 1.0
    R = (S @ S.T) / GS  # fused group-average + broadcast
    pk1 = np.concatenate(
        [R, arr_c(norm_scale), arr_c(norm_bias), arr_c(bu)], axis=1)
    pk2 = np.concatenate(
        [np.ones(512, np.float32), bvo.reshape(C)]).reshape(1, -1)
    common = {
        "MTb": arr_w(M).astype(NP_BF16),
        "wov8T": _q8(arr_w(Wov) * SPLIT),
        "pk1": np.ascontiguousarray(pk1),
        "pk2": np.ascontiguousarray(pk2),
    }

    # x: (B, C, H, W) -> per core [NB, p, ct, n]
    xf = x.reshape(B, C, N).reshape(B, CT, 128, N).transpose(0, 2, 1, 3)
    in_maps = [
        {**common, "x": np.ascontiguousarray(xf[i * NB:(i + 1) * NB])}
        for i in range(NCORES)
    ]
    flags = (bool(np.any(bq != 0.0)), bool(np.any(bk != 0.0)),
             bool(np.any(bv != 0.0)), bool(np.any(bo != 0.0)))
    return in_maps, flags


def assemble(results):
    y = np.empty((B, C, N), np.float32)
    for i in range(NCORES):
        yc = results[i]["y"]  # [NB, 128, CT, N]
        y[i * NB:(i + 1) * NB] = (
            yc.transpose(0, 2, 1, 3).reshape(NB, C, N))
    return y.reshape(B, C, H, W)


def kernel(x, norm_scale, norm_bias, wq, bq, wk, bk, wv, bv, wo, bo):
    in_maps, flags = prepare(x, norm_scale, norm_bias, wq, bq,
                             wk, bk, wv, bv, wo, bo)
    nc = _get_nc(*flags)
    res = run_bass_kernel_spmd(nc, in_maps, list(range(NCORES)))
    return assemble(res.results)
